# revision 10
# baseline (speedup 1.0000x reference)
"""Trainium2 Bass kernel for streaming dot-product attention with alpha decay.

Math: with e~_s = alpha^{-s} exp(qk_s) the scan becomes a prefix sum computed
as a triangular-ones matmul; QKV_0/Z_0 enter via row-0 fold / K=1 matmul.

v6 strategy ([d, n]-major layout):
- Everything downstream of R uses [t, d, n] ordering so the 1/den operand
  broadcasts over the MIDDLE dim (innermost step-1) -> DVE 2x mode for the
  fp16 evacuated divides.  R'[s,d,n] = v[s,d]*e~[s,n]: v expanded densely on
  ACT (Copy), e~ middle-broadcast -> 2x R-build too.
- p0 computed transposed ([D+1, N1], vin as stationary) so the Z0 row is
  directly a [1, N1] SBUF row (no flatten DMA) and the QKV0 fold lands
  n-contiguous.  out0 = QKV0/Z0 finishes on host from the tiny shipped p0.
- fp16 output, DRAM [BL, T, D, N1] b-major; one 1 MB DMA per b; host
  transposes/casts back.
- divide pairs [T,1024]: 2 on DVE direct (1x fp32-psum), 2 via ACT evac +
  DVE fp16 TT at 2x.  R via ACT vexp route for 5 b's, DVE 1x direct for 3.
"""

import math
from contextlib import ExitStack

import numpy as np

import concourse.bass as bass
import concourse.bacc as bacc
import concourse.tile as tile
from concourse import mybir
from concourse.bass_utils import run_bass_kernel_spmd

ALPHA = 0.99
B, N1, N2, D, T = 64, 64, 512, 64, 128
NCORES = 8
BL = B // NCORES
F32 = mybir.dt.float32
F16 = mybir.dt.float16
Exp = mybir.ActivationFunctionType.Exp
Copy = mybir.ActivationFunctionType.Copy


def _build():
    nc = bacc.Bacc("TRN2", target_bir_lowering=False, debug=False)

    qT_d = nc.dram_tensor("qT", [BL, D, N1], F16, kind="ExternalInput")
    kT_d = nc.dram_tensor("kT", [BL, D, N2], F16, kind="ExternalInput")
    vin_d = nc.dram_tensor("vin", [BL, 4, 128, D + 1], F16, kind="ExternalInput")
    ksT_d = nc.dram_tensor("ksT", [BL, D, T], F16, kind="ExternalInput")
    vst_d = nc.dram_tensor("vst", [BL, T, D], F16, kind="ExternalInput")
    tri_d = nc.dram_tensor("tri", [T, T], F16, kind="ExternalInput")
    sb_d = nc.dram_tensor("sbias", [T, 1], F32, kind="ExternalInput")
    out_d = nc.dram_tensor("out", [BL, T, D, N1], F16, kind="ExternalOutput")
    p0_d = nc.dram_tensor("p0out", [D + 1, BL, N1], F16, kind="ExternalOutput")

    VEXP_R = {1, 3, 5, 7}  # R via ACT vexp + DVE 2x; others DVE 1x direct

    with tile.TileContext(nc) as tc, ExitStack() as ctx:
        consts = ctx.enter_context(tc.tile_pool(name="consts", bufs=1))
        inbuf = ctx.enter_context(tc.tile_pool(name="inbuf", bufs=1))
        small = ctx.enter_context(tc.tile_pool(name="small", bufs=6))
        ebuf = ctx.enter_context(tc.tile_pool(name="ebuf", bufs=2))
        rbuf = ctx.enter_context(tc.tile_pool(name="rbuf", bufs=3))
        obuf = ctx.enter_context(tc.tile_pool(name="obuf", bufs=3))
        psum = ctx.enter_context(tc.tile_pool(name="psum", bufs=1, space="PSUM"))

        tri = consts.tile([T, T], F16)
        nc.sync.dma_start(out=tri[:], in_=tri_d[:])
        sbias = consts.tile([T, 1], F32)
        nc.sync.dma_start(out=sbias[:], in_=sb_d[:])

        qT_all = inbuf.tile([D, BL, N1], F16)
        kT_all = inbuf.tile([D, BL, N2], F16)
        ksT_all = inbuf.tile([D, BL, T], F16)
        vin_all = inbuf.tile([128, BL, 4, D + 1], F16)
        vst_all = inbuf.tile([T, BL, D], F16)
        p0all = inbuf.tile([D + 1, BL, N1], F16)

        # b0/b1 input slices land first so compute starts early; rest bulk
        nc.sync.dma_start(out=qT_all[:], in_=qT_d.rearrange("b d n -> d b n"))
        for b in (0, 1):
            nc.sync.dma_start(out=kT_all[:, b, :], in_=kT_d[b])
            nc.gpsimd.dma_start(
                out=vin_all[:, b, :, :], in_=vin_d[b].rearrange("c p e -> p c e")
            )
            nc.sync.dma_start(out=ksT_all[:, b, :], in_=ksT_d[b])
            nc.gpsimd.dma_start(out=vst_all[:, b, :], in_=vst_d[b])
        rs = slice(2, BL)
        nc.sync.dma_start(out=kT_all[:, rs, :], in_=kT_d[rs].rearrange("b d m -> d b m"))
        nc.gpsimd.dma_start(
            out=vin_all[:, rs, :, :], in_=vin_d[rs].rearrange("b c p e -> p b c e")
        )
        nc.sync.dma_start(out=ksT_all[:, rs, :], in_=ksT_d[rs].rearrange("b d t -> d b t"))
        nc.gpsimd.dma_start(out=vst_all[:, rs, :], in_=vst_d[rs].rearrange("b t d -> t b d"))

        for b in range(BL):
            qT = qT_all[:, b, :]
            use_vexp = b in VEXP_R

            # init attention logits: qk[c] [128, 64] = kT_c^T q
            qk_ps = psum.tile([128, 4, N1], F32, tag="pqk", bufs=2)
            for c in range(4):
                nc.tensor.matmul(
                    qk_ps[:, c, :], kT_all[:, b, 128 * c : 128 * (c + 1)], qT,
                    start=True, stop=True,
                )
            qke = small.tile([128, 4, N1], F16, tag="qke")
            nc.scalar.activation(qke[:], qk_ps[:], Exp)

            # transposed [QKV0|Z0]: p0T [D+1, N1] (vin stationary)
            p0T = psum.tile([D + 1, N1], F32, tag="ptr", bufs=2)
            for c in range(4):
                nc.tensor.matmul(
                    p0T[:], vin_all[:, b, c, :], qke[:, c, :],
                    start=(c == 0), stop=(c == 3),
                )
            # fp16 copy straight into the staging tile shipped to host
            nc.scalar.activation(p0all[:, b, :], p0T[:], Copy)

            # stream logits ps_s [T, N1]
            ps_s = psum.tile([T, N1], F32, tag="pqk", bufs=2)
            nc.tensor.matmul(ps_s[:], ksT_all[:, b, :], qT, start=True, stop=True)

            # plain eb first: den/reciprocal path never waits on vexp
            eb = small.tile([T, N1], F16, tag="eb")
            nc.scalar.activation(eb[:], ps_s[:], Exp, bias=sbias[:], scale=1.0)

            # Z0 row to partition 0 for the K=1 den fold
            z0f = small.tile([1, N1], F16, tag="z0f")
            nc.gpsimd.dma_start(out=z0f[:], in_=p0all[D : D + 1, b, :])

            # den + reciprocal (critical path to every divide)
            pden = psum.tile([T, N1], F32, tag="pqk", bufs=2)
            nc.tensor.matmul(pden[:], tri[:], eb[:], start=True, stop=False)
            nc.tensor.matmul(pden[:], tri[0:1, :], z0f[:], start=False, stop=True)
            r_t = small.tile([T, N1], F32, tag="r")
            nc.vector.reciprocal(r_t[:], pden[:])
            rh = small.tile([T, N1], F16, tag="rh")
            nc.scalar.activation(rh[:], r_t[:], Copy)

            # R'[s,d,n] = v[s,d] * e~[s,n]
            R_t = rbuf.tile([T, D, N1], F16, tag="R")
            if use_vexp:
                vexp = ebuf.tile([T, D, N1], F16, tag="vexp")
                nc.scalar.activation(
                    vexp[:],
                    vst_all[:, b, :, None].broadcast_to([T, D, N1]),
                    Copy,
                )
                nc.vector.tensor_mul(
                    R_t[:],
                    vexp[:],
                    eb[:, None, :].broadcast_to([T, D, N1]),
                )
            else:
                nc.vector.tensor_mul(
                    R_t[:],
                    vst_all[:, b, :, None].broadcast_to([T, D, N1]),
                    eb[:, None, :].broadcast_to([T, D, N1]),
                )
            nc.gpsimd.dma_start(
                out=R_t[0:1, :, :], in_=p0all[0:D, b, None, :],
                accum_op=mybir.AluOpType.add,
            )

            # numerator matmuls in pairs -> [T, 2, 512] psum; divide per pair
            obig = obuf.tile([T, D, N1], F16, tag="obig")
            for pair in range(4):
                pnum = psum.tile([T, 2, 512], F32, tag="pbig", bufs=2)
                for h in range(2):
                    c = 2 * pair + h
                    nc.tensor.matmul(
                        pnum[:, h, :], tri[:],
                        R_t[:, 8 * c : 8 * (c + 1), :].rearrange(
                            "t d n -> t (d n)"
                        ),
                        start=True, stop=True,
                    )
                ds = slice(16 * pair, 16 * (pair + 1))
                pview = pnum[:].rearrange("t h (d n) -> t (h d) n", n=N1)
                if pair < 2:
                    nc.vector.tensor_mul(
                        obig[:, ds, :],
                        pview,
                        r_t[:, None, :].broadcast_to([T, 16, N1]),
                    )
                else:
                    numh = small.tile([T, 16, N1], F16, tag="numh")
                    nc.scalar.activation(numh[:], pview, Copy)
                    nc.vector.tensor_mul(
                        obig[:, ds, :],
                        numh[:],
                        rh[:, None, :].broadcast_to([T, 16, N1]),
                    )

            nc.sync.dma_start(
                out=out_d[b], in_=obig[:].rearrange("t d n -> t (d n)")
            )

        nc.sync.dma_start(out=p0_d[:], in_=p0all[:])

    nc.compile()
    return nc


_CACHE = {}


def _get_nc():
    if "nc" not in _CACHE:
        _CACHE["nc"] = _build()
    return _CACHE["nc"]


def _in_maps(q, k_init, v_init, k_stream, v_stream):
    q = np.asarray(q, np.float32).astype(np.float16)
    k_init = np.asarray(k_init, np.float32).astype(np.float16)
    v_init = np.asarray(v_init, np.float32).astype(np.float16)
    k_stream = np.asarray(k_stream, np.float32).astype(np.float16)
    v_stream = np.asarray(v_stream, np.float32).astype(np.float16)

    qT = np.ascontiguousarray(q.transpose(0, 2, 1))            # [B, D, N1]
    kT = np.ascontiguousarray(k_init.transpose(0, 2, 1))       # [B, D, N2]
    vin = np.ones((B, 4, 128, D + 1), np.float16)
    vin[:, :, :, 0:D] = v_init.reshape(B, 4, 128, D)
    ksT = np.ascontiguousarray(k_stream.transpose(1, 2, 0))    # [B, D, T]
    vst = np.ascontiguousarray(v_stream.transpose(1, 0, 2))    # [B, T, D]

    tri = np.triu(np.ones((T, T), np.float32)).astype(np.float16)
    sbias = (np.arange(1, T + 1, dtype=np.float64) * (-math.log(ALPHA))).astype(
        np.float32
    ).reshape(T, 1)
    maps = []
    for i in range(NCORES):
        sl = slice(i * BL, (i + 1) * BL)
        maps.append(
            dict(
                qT=np.ascontiguousarray(qT[sl]),
                kT=np.ascontiguousarray(kT[sl]),
                vin=np.ascontiguousarray(vin[sl]),
                ksT=np.ascontiguousarray(ksT[sl]),
                vst=np.ascontiguousarray(vst[sl]),
                tri=tri,
                sbias=sbias,
            )
        )
    return maps


def run(q, k_init, v_init, attn_mask, k_stream, v_stream, trace=False, **trace_kw):
    """Run on hardware; returns (output, BassKernelResults)."""
    nc = _get_nc()
    maps = _in_maps(q, k_init, v_init, k_stream, v_stream)
    res = run_bass_kernel_spmd(nc, maps, list(range(NCORES)), trace=trace, **trace_kw)
    out = np.empty((T + 1, B, N1, D), np.float32)
    for i in range(NCORES):
        sl = slice(i * BL, (i + 1) * BL)
        # stream rows: [BL, T, D, N1] -> [T, BL, N1, D]
        out[1:, sl] = res.results[i]["out"].transpose(1, 0, 3, 2).astype(np.float32)
        # row 0 from shipped [QKV0|Z0]: p0 [D+1, BL, N1]
        p0 = res.results[i]["p0out"].astype(np.float32)
        out[0, sl] = (p0[0:D] / p0[D : D + 1]).transpose(1, 2, 0)
    return out, res


def kernel(q, k_init, v_init, attn_mask, k_stream, v_stream):
    out, _ = run(q, k_init, v_init, attn_mask, k_stream, v_stream, trace=False)
    return out


# revision 11
# speedup vs baseline: 1.0450x; 1.0450x over previous
"""Trainium2 Bass kernel for streaming dot-product attention with alpha decay.

Math: with e~_s = alpha^{-s} exp(qk_s) the scan becomes a prefix sum computed
as a triangular-ones matmul; QKV_0/Z_0 enter via row-0 fold / K=1 matmul.

v3 strategy:
- Host pre-transposes all inputs (qT/kT/ksT/vst; vin chunked with baked
  ones-column); no device transposes at all.
- fp16 output, DRAM [BL, T+1, N1, D] b-major; one 1 MB DMA per b.
- Per-b critical path kept short: plain eb exp first (den/recip path), then
  ebexp (exp fused with d-broadcast) for DVE 2x R-build; R built in two
  n-halves so pnum matmuls start after half 1; QKV0 fold DMA also halved.
- Work split: R on DVE (2x via ebexp) for 6 b's, GpSimd 1x for 2 b's;
  divide pairs [T,1024]: 3 on DVE direct (1x fp32-psum), 1 via ACT evac +
  GpSimd fp16 TT.  Small copies (qkv0_h, zcol_h) on ACT.
"""

import math
from contextlib import ExitStack

import numpy as np

import concourse.bass as bass
import concourse.bacc as bacc
import concourse.tile as tile
from concourse import mybir
from concourse.bass_utils import run_bass_kernel_spmd

ALPHA = 0.99
B, N1, N2, D, T = 64, 64, 512, 64, 128
NCORES = 8
BL = B // NCORES
F32 = mybir.dt.float32
F16 = mybir.dt.float16
Exp = mybir.ActivationFunctionType.Exp
Copy = mybir.ActivationFunctionType.Copy


def _build():
    nc = bacc.Bacc("TRN2", target_bir_lowering=False, debug=False)

    qT_d = nc.dram_tensor("qT", [BL, D, N1], F16, kind="ExternalInput")
    kT_d = nc.dram_tensor("kT", [BL, D, N2], F16, kind="ExternalInput")
    vin_d = nc.dram_tensor("vin", [BL, 4, 128, D + 1], F16, kind="ExternalInput")
    ksT_d = nc.dram_tensor("ksT", [BL, D, T], F16, kind="ExternalInput")
    vst_d = nc.dram_tensor("vst", [BL, T, D], F16, kind="ExternalInput")
    tri_d = nc.dram_tensor("tri", [T, T], F16, kind="ExternalInput")
    sb_d = nc.dram_tensor("sbias", [T, 1], F32, kind="ExternalInput")
    out_d = nc.dram_tensor("out", [BL, T + 1, N1, D], F16, kind="ExternalOutput")

    EBEXP_R = {1, 3, 4, 6, 7}  # R via ACT ebexp + DVE 2x; others DVE 1x direct

    with tile.TileContext(nc) as tc, ExitStack() as ctx:
        consts = ctx.enter_context(tc.tile_pool(name="consts", bufs=1))
        inbuf = ctx.enter_context(tc.tile_pool(name="inbuf", bufs=1))
        small = ctx.enter_context(tc.tile_pool(name="small", bufs=6))
        ebuf = ctx.enter_context(tc.tile_pool(name="ebuf", bufs=2))
        rbuf = ctx.enter_context(tc.tile_pool(name="rbuf", bufs=3))
        obuf = ctx.enter_context(tc.tile_pool(name="obuf", bufs=3))
        psum = ctx.enter_context(tc.tile_pool(name="psum", bufs=1, space="PSUM"))

        tri = consts.tile([T, T], F16)
        nc.sync.dma_start(out=tri[:], in_=tri_d[:])
        sbias = consts.tile([T, 1], F32)
        nc.sync.dma_start(out=sbias[:], in_=sb_d[:])

        qT_all = inbuf.tile([D, BL, N1], F16)
        kT_all = inbuf.tile([D, BL, N2], F16)
        ksT_all = inbuf.tile([D, BL, T], F16)
        vin_all = inbuf.tile([128, BL, 4, D + 1], F16)
        vst_all = inbuf.tile([T, BL, D], F16)
        o0all = inbuf.tile([N1, BL, D], F16)

        # b0/b1 input slices land first so compute starts early; rest bulk
        nc.sync.dma_start(out=qT_all[:], in_=qT_d.rearrange("b d n -> d b n"))
        for b in (0, 1):
            e1 = nc.sync if b % 2 == 0 else nc.scalar
            e2 = nc.scalar if b % 2 == 0 else nc.sync
            e1.dma_start(out=kT_all[:, b, :], in_=kT_d[b])
            e2.dma_start(
                out=vin_all[:, b, :, :], in_=vin_d[b].rearrange("c p e -> p c e")
            )
            e1.dma_start(out=ksT_all[:, b, :], in_=ksT_d[b])
            e2.dma_start(out=vst_all[:, b, :], in_=vst_d[b])
        rs = slice(2, BL)
        nc.sync.dma_start(out=kT_all[:, rs, :], in_=kT_d[rs].rearrange("b d m -> d b m"))
        nc.scalar.dma_start(
            out=vin_all[:, rs, :, :], in_=vin_d[rs].rearrange("b c p e -> p b c e")
        )
        nc.sync.dma_start(out=ksT_all[:, rs, :], in_=ksT_d[rs].rearrange("b d t -> d b t"))
        nc.scalar.dma_start(out=vst_all[:, rs, :], in_=vst_d[rs].rearrange("b t d -> t b d"))

        for b in range(BL):
            qT = qT_all[:, b, :]
            use_ebexp = b in EBEXP_R

            # init attention logits: qk[c] [128, 64] = kT_c^T q
            qk_ps = psum.tile([128, 4, N1], F32, tag="pqk", bufs=2)
            for c in range(4):
                nc.tensor.matmul(
                    qk_ps[:, c, :], kT_all[:, b, 128 * c : 128 * (c + 1)], qT,
                    start=True, stop=True,
                )
            qke = small.tile([128, 4, N1], F16, tag="qke")
            nc.scalar.activation(qke[:], qk_ps[:], Exp)

            # [QKV_0 | Z_0]: p0 [64, 65]
            p0 = psum.tile([N1, D + 1], F32, tag="ptr", bufs=2)
            for c in range(4):
                nc.tensor.matmul(
                    p0[:], qke[:, c, :], vin_all[:, b, c, :],
                    start=(c == 0), stop=(c == 3),
                )

            # stream logits ps_s [T, N1]
            ps_s = psum.tile([T, N1], F32, tag="pqk", bufs=2)
            nc.tensor.matmul(ps_s[:], ksT_all[:, b, :], qT, start=True, stop=True)

            # plain eb first: den/reciprocal path never waits on ebexp
            eb = small.tile([T, N1], F16, tag="eb")
            nc.scalar.activation(eb[:], ps_s[:], Exp, bias=sbias[:], scale=1.0)

            # fp16 copy of [QKV0|Z0] on ACT; z0f flatten on gpsimd queue
            p0h = small.tile([N1, D + 1], F16, tag="p0h")
            nc.scalar.activation(p0h[:], p0[:], Copy)
            z0f = small.tile([1, N1], F16, tag="z0f")
            nc.gpsimd.dma_start(out=z0f[:], in_=p0h[:, D : D + 1])

            # out0 = QKV_0/Z_0 into o0all
            rz = small.tile([N1, 1], F32, tag="rz")
            nc.vector.reciprocal(rz[:], p0[:, D : D + 1])
            nc.vector.tensor_scalar_mul(o0all[:, b, :], p0[:, 0:D], rz[:])

            # den + reciprocal (critical path to every divide)
            pden = psum.tile([T, N1], F32, tag="pqk", bufs=2)
            nc.tensor.matmul(pden[:], tri[:], eb[:], start=True, stop=False)
            nc.tensor.matmul(pden[:], tri[0:1, :], z0f[:], start=False, stop=True)
            r_t = small.tile([T, N1], F32, tag="r")
            nc.vector.reciprocal(r_t[:], pden[:])

            # R[s,n,d] = e~[s,n] * v[s,d]
            R_t = rbuf.tile([T, N1, D], F16, tag="R")
            if use_ebexp:
                ebexp = ebuf.tile([T, N1, D], F16, tag="ebexp")
                nc.scalar.activation(
                    ebexp[:],
                    ps_s[:, :, None].broadcast_to([T, N1, D]),
                    Exp, bias=sbias[:], scale=1.0,
                )
                nc.vector.tensor_mul(
                    R_t[:],
                    ebexp[:],
                    vst_all[:, b, None, :].broadcast_to([T, N1, D]),
                )
            else:
                nc.vector.tensor_mul(
                    R_t[:],
                    eb[:, :, None].broadcast_to([T, N1, D]),
                    vst_all[:, b, None, :].broadcast_to([T, N1, D]),
                )
            nc.gpsimd.dma_start(
                out=R_t[0:1, :, :], in_=p0h[:, None, 0:D],
                accum_op=mybir.AluOpType.add,
            )

            # numerator matmuls in pairs -> [T, 2, 512] psum; divide per pair
            obig = obuf.tile([T, N1, D], F16, tag="obig")
            for pair in range(4):
                pnum = psum.tile([T, 2, 512], F32, tag="pbig", bufs=2)
                for h in range(2):
                    c = 2 * pair + h
                    nc.tensor.matmul(
                        pnum[:, h, :], tri[:],
                        R_t[:, 8 * c : 8 * (c + 1), :].rearrange(
                            "t n d -> t (n d)"
                        ),
                        start=True, stop=True,
                    )
                ns = slice(16 * pair, 16 * (pair + 1))
                pview = pnum[:].rearrange("t h (n d) -> t (h n) d", d=D)
                if pair < 2:
                    nc.vector.tensor_mul(
                        obig[:, ns, :],
                        pview,
                        r_t[:, ns, None].broadcast_to([T, 16, D]),
                    )
                else:
                    numh = small.tile([T, 16, D], F16, tag="numh")
                    nc.scalar.activation(numh[:], pview, Copy)
                    nc.vector.tensor_mul(
                        obig[:, ns, :],
                        numh[:],
                        r_t[:, ns, None].broadcast_to([T, 16, D]),
                    )

            eng = nc.sync if b % 2 == 0 else nc.scalar
            eng.dma_start(
                out=out_d[b, 1:], in_=obig[:].rearrange("t n d -> t (n d)")
            )

        nc.sync.dma_start(
            out=out_d[:, 0].rearrange("b n d -> n b d"), in_=o0all[:]
        )

    nc.compile()
    return nc


_CACHE = {}


def _get_nc():
    if "nc" not in _CACHE:
        _CACHE["nc"] = _build()
    return _CACHE["nc"]


def _in_maps(q, k_init, v_init, k_stream, v_stream):
    q = np.asarray(q, np.float32).astype(np.float16)
    k_init = np.asarray(k_init, np.float32).astype(np.float16)
    v_init = np.asarray(v_init, np.float32).astype(np.float16)
    k_stream = np.asarray(k_stream, np.float32).astype(np.float16)
    v_stream = np.asarray(v_stream, np.float32).astype(np.float16)

    qT = np.ascontiguousarray(q.transpose(0, 2, 1))            # [B, D, N1]
    kT = np.ascontiguousarray(k_init.transpose(0, 2, 1))       # [B, D, N2]
    vin = np.ones((B, 4, 128, D + 1), np.float16)
    vin[:, :, :, 0:D] = v_init.reshape(B, 4, 128, D)
    ksT = np.ascontiguousarray(k_stream.transpose(1, 2, 0))    # [B, D, T]
    vst = np.ascontiguousarray(v_stream.transpose(1, 0, 2))    # [B, T, D]

    tri = np.triu(np.ones((T, T), np.float32)).astype(np.float16)
    sbias = (np.arange(1, T + 1, dtype=np.float64) * (-math.log(ALPHA))).astype(
        np.float32
    ).reshape(T, 1)
    maps = []
    for i in range(NCORES):
        sl = slice(i * BL, (i + 1) * BL)
        maps.append(
            dict(
                qT=np.ascontiguousarray(qT[sl]),
                kT=np.ascontiguousarray(kT[sl]),
                vin=np.ascontiguousarray(vin[sl]),
                ksT=np.ascontiguousarray(ksT[sl]),
                vst=np.ascontiguousarray(vst[sl]),
                tri=tri,
                sbias=sbias,
            )
        )
    return maps


def run(q, k_init, v_init, attn_mask, k_stream, v_stream, trace=False, **trace_kw):
    """Run on hardware; returns (output, BassKernelResults)."""
    nc = _get_nc()
    maps = _in_maps(q, k_init, v_init, k_stream, v_stream)
    res = run_bass_kernel_spmd(nc, maps, list(range(NCORES)), trace=trace, **trace_kw)
    out = np.concatenate(
        [res.results[i]["out"].transpose(1, 0, 2, 3) for i in range(NCORES)],
        axis=1,
    ).astype(np.float32)
    return out, res


def kernel(q, k_init, v_init, attn_mask, k_stream, v_stream):
    out, _ = run(q, k_init, v_init, attn_mask, k_stream, v_stream, trace=False)
    return out


# revision 12
# speedup vs baseline: 1.0972x; 1.0500x over previous
"""Trainium2 Bass kernel for streaming dot-product attention with alpha decay.

Math: with e~_s = alpha^{-s} exp(qk_s) the scan becomes a prefix sum computed
as a triangular-ones matmul; QKV_0/Z_0 enter via row-0 fold / K=1 matmul.

v3 strategy:
- Host pre-transposes all inputs (qT/kT/ksT/vst; vin chunked with baked
  ones-column); no device transposes at all.
- fp16 output, DRAM [BL, T+1, N1, D] b-major; one 1 MB DMA per b.
- Per-b critical path kept short: plain eb exp first (den/recip path), then
  ebexp (exp fused with d-broadcast) for DVE 2x R-build; R built in two
  n-halves so pnum matmuls start after half 1; QKV0 fold DMA also halved.
- Work split: R on DVE (2x via ebexp) for 6 b's, GpSimd 1x for 2 b's;
  divide pairs [T,1024]: 3 on DVE direct (1x fp32-psum), 1 via ACT evac +
  GpSimd fp16 TT.  Small copies (qkv0_h, zcol_h) on ACT.
"""

import math
from contextlib import ExitStack

import numpy as np

import concourse.bass as bass
import concourse.bacc as bacc
import concourse.tile as tile
from concourse import mybir
from concourse.bass_utils import run_bass_kernel_spmd

ALPHA = 0.99
B, N1, N2, D, T = 64, 64, 512, 64, 128
NCORES = 8
BL = B // NCORES
F32 = mybir.dt.float32
F16 = mybir.dt.float16
Exp = mybir.ActivationFunctionType.Exp
Copy = mybir.ActivationFunctionType.Copy


def _build():
    nc = bacc.Bacc("TRN2", target_bir_lowering=False, debug=False)

    qT_d = nc.dram_tensor("qT", [BL, D, N1], F16, kind="ExternalInput")
    kT_d = nc.dram_tensor("kT", [BL, D, N2], F16, kind="ExternalInput")
    vin_d = nc.dram_tensor("vin", [BL, 4, 128, D + 1], F16, kind="ExternalInput")
    ksT_d = nc.dram_tensor("ksT", [BL, D, T], F16, kind="ExternalInput")
    vst_d = nc.dram_tensor("vst", [BL, T, D], F16, kind="ExternalInput")
    tri_d = nc.dram_tensor("tri", [T, T], F16, kind="ExternalInput")
    sb_d = nc.dram_tensor("sbias", [T, 1], F32, kind="ExternalInput")
    out_d = nc.dram_tensor("out", [BL, T + 1, N1, D], F16, kind="ExternalOutput")

    EBEXP_R = set(range(BL))  # all R via ACT ebexp + DVE 2x

    with tile.TileContext(nc) as tc, ExitStack() as ctx:
        consts = ctx.enter_context(tc.tile_pool(name="consts", bufs=1))
        inbuf = ctx.enter_context(tc.tile_pool(name="inbuf", bufs=1))
        small = ctx.enter_context(tc.tile_pool(name="small", bufs=6))
        ebuf = ctx.enter_context(tc.tile_pool(name="ebuf", bufs=2))
        rbuf = ctx.enter_context(tc.tile_pool(name="rbuf", bufs=3))
        obuf = ctx.enter_context(tc.tile_pool(name="obuf", bufs=3))
        psum = ctx.enter_context(tc.tile_pool(name="psum", bufs=1, space="PSUM"))

        tri = consts.tile([T, T], F16)
        nc.sync.dma_start(out=tri[:], in_=tri_d[:])
        sbias = consts.tile([T, 1], F32)
        nc.sync.dma_start(out=sbias[:], in_=sb_d[:])

        qT_all = inbuf.tile([D, BL, N1], F16)
        kT_all = inbuf.tile([D, BL, N2], F16)
        ksT_all = inbuf.tile([D, BL, T], F16)
        vin_all = inbuf.tile([128, BL, 4, D + 1], F16)
        vst_all = inbuf.tile([T, BL, D], F16)
        o0all = inbuf.tile([N1, BL, D], F16)

        # b0/b1 input slices land first so compute starts early; rest bulk
        nc.sync.dma_start(out=qT_all[:], in_=qT_d.rearrange("b d n -> d b n"))
        for b in (0, 1):
            e1 = nc.sync if b % 2 == 0 else nc.scalar
            e2 = nc.scalar if b % 2 == 0 else nc.sync
            e1.dma_start(out=kT_all[:, b, :], in_=kT_d[b])
            e2.dma_start(
                out=vin_all[:, b, :, :], in_=vin_d[b].rearrange("c p e -> p c e")
            )
            e1.dma_start(out=ksT_all[:, b, :], in_=ksT_d[b])
            e2.dma_start(out=vst_all[:, b, :], in_=vst_d[b])
        rs = slice(2, BL)
        nc.sync.dma_start(out=kT_all[:, rs, :], in_=kT_d[rs].rearrange("b d m -> d b m"))
        nc.scalar.dma_start(
            out=vin_all[:, rs, :, :], in_=vin_d[rs].rearrange("b c p e -> p b c e")
        )
        nc.sync.dma_start(out=ksT_all[:, rs, :], in_=ksT_d[rs].rearrange("b d t -> d b t"))
        nc.scalar.dma_start(out=vst_all[:, rs, :], in_=vst_d[rs].rearrange("b t d -> t b d"))

        for b in range(BL):
            qT = qT_all[:, b, :]
            use_ebexp = b in EBEXP_R

            # init attention logits: qk[c] [128, 64] = kT_c^T q
            qk_ps = psum.tile([128, 4, N1], F32, tag="pqk", bufs=2)
            for c in range(4):
                nc.tensor.matmul(
                    qk_ps[:, c, :], kT_all[:, b, 128 * c : 128 * (c + 1)], qT,
                    start=True, stop=True,
                )
            qke = small.tile([128, 4, N1], F16, tag="qke")
            nc.scalar.activation(qke[:], qk_ps[:], Exp)

            # [QKV_0 | Z_0]: p0 [64, 65]
            p0 = psum.tile([N1, D + 1], F32, tag="ptr", bufs=2)
            for c in range(4):
                nc.tensor.matmul(
                    p0[:], qke[:, c, :], vin_all[:, b, c, :],
                    start=(c == 0), stop=(c == 3),
                )

            # stream logits ps_s [T, N1]
            ps_s = psum.tile([T, N1], F32, tag="pqk", bufs=2)
            nc.tensor.matmul(ps_s[:], ksT_all[:, b, :], qT, start=True, stop=True)

            # plain eb first: den/reciprocal path never waits on ebexp
            eb = small.tile([T, N1], F16, tag="eb")
            nc.scalar.activation(eb[:], ps_s[:], Exp, bias=sbias[:], scale=1.0)

            # fp16 copy of [QKV0|Z0] on ACT; z0f flatten on gpsimd queue
            p0h = small.tile([N1, D + 1], F16, tag="p0h")
            nc.scalar.activation(p0h[:], p0[:], Copy)
            z0f = small.tile([1, N1], F16, tag="z0f")
            nc.gpsimd.dma_start(out=z0f[:], in_=p0h[:, D : D + 1])

            # out0 = QKV_0/Z_0 into o0all (multiply on ACT via scale)
            rz = small.tile([N1, 1], F32, tag="rz")
            nc.vector.reciprocal(rz[:], p0[:, D : D + 1])
            nc.scalar.activation(o0all[:, b, :], p0[:, 0:D], Copy, scale=rz[:])

            # den + reciprocal (critical path to every divide)
            pden = psum.tile([T, N1], F32, tag="pqk", bufs=2)
            nc.tensor.matmul(pden[:], tri[:], eb[:], start=True, stop=False)
            nc.tensor.matmul(pden[:], tri[0:1, :], z0f[:], start=False, stop=True)
            r_t = small.tile([T, N1], F32, tag="r")
            nc.vector.reciprocal(r_t[:], pden[:])

            # R[s,n,d] = e~[s,n] * v[s,d]
            R_t = rbuf.tile([T, N1, D], F16, tag="R")
            if use_ebexp:
                ebexp = ebuf.tile([T, N1, D], F16, tag="ebexp")
                nc.scalar.activation(
                    ebexp[:],
                    ps_s[:, :, None].broadcast_to([T, N1, D]),
                    Exp, bias=sbias[:], scale=1.0,
                )
                nc.vector.tensor_mul(
                    R_t[:],
                    ebexp[:],
                    vst_all[:, b, None, :].broadcast_to([T, N1, D]),
                )
            else:
                nc.vector.tensor_mul(
                    R_t[:],
                    eb[:, :, None].broadcast_to([T, N1, D]),
                    vst_all[:, b, None, :].broadcast_to([T, N1, D]),
                )
            nc.gpsimd.dma_start(
                out=R_t[0:1, :, :], in_=p0h[:, None, 0:D],
                accum_op=mybir.AluOpType.add,
            )

            # numerator matmuls in pairs -> [T, 2, 512] psum; divide per pair
            obig = obuf.tile([T, N1, D], F16, tag="obig")
            for pair in range(4):
                pnum = psum.tile([T, 2, 512], F32, tag="pbig", bufs=2)
                for h in range(2):
                    c = 2 * pair + h
                    nc.tensor.matmul(
                        pnum[:, h, :], tri[:],
                        R_t[:, 8 * c : 8 * (c + 1), :].rearrange(
                            "t n d -> t (n d)"
                        ),
                        start=True, stop=True,
                    )
                ns = slice(16 * pair, 16 * (pair + 1))
                pview = pnum[:].rearrange("t h (n d) -> t (h n) d", d=D)
                nc.vector.tensor_mul(
                    obig[:, ns, :],
                    pview,
                    r_t[:, ns, None].broadcast_to([T, 16, D]),
                )

            eng = nc.sync if b % 2 == 0 else nc.scalar
            eng.dma_start(
                out=out_d[b, 1:], in_=obig[:].rearrange("t n d -> t (n d)")
            )

        nc.sync.dma_start(
            out=out_d[:, 0].rearrange("b n d -> n b d"), in_=o0all[:]
        )

    nc.compile()
    return nc


_CACHE = {}


def _get_nc():
    if "nc" not in _CACHE:
        _CACHE["nc"] = _build()
    return _CACHE["nc"]


def _in_maps(q, k_init, v_init, k_stream, v_stream):
    q = np.asarray(q, np.float32).astype(np.float16)
    k_init = np.asarray(k_init, np.float32).astype(np.float16)
    v_init = np.asarray(v_init, np.float32).astype(np.float16)
    k_stream = np.asarray(k_stream, np.float32).astype(np.float16)
    v_stream = np.asarray(v_stream, np.float32).astype(np.float16)

    qT = np.ascontiguousarray(q.transpose(0, 2, 1))            # [B, D, N1]
    kT = np.ascontiguousarray(k_init.transpose(0, 2, 1))       # [B, D, N2]
    vin = np.ones((B, 4, 128, D + 1), np.float16)
    vin[:, :, :, 0:D] = v_init.reshape(B, 4, 128, D)
    ksT = np.ascontiguousarray(k_stream.transpose(1, 2, 0))    # [B, D, T]
    vst = np.ascontiguousarray(v_stream.transpose(1, 0, 2))    # [B, T, D]

    tri = np.triu(np.ones((T, T), np.float32)).astype(np.float16)
    sbias = (np.arange(1, T + 1, dtype=np.float64) * (-math.log(ALPHA))).astype(
        np.float32
    ).reshape(T, 1)
    maps = []
    for i in range(NCORES):
        sl = slice(i * BL, (i + 1) * BL)
        maps.append(
            dict(
                qT=np.ascontiguousarray(qT[sl]),
                kT=np.ascontiguousarray(kT[sl]),
                vin=np.ascontiguousarray(vin[sl]),
                ksT=np.ascontiguousarray(ksT[sl]),
                vst=np.ascontiguousarray(vst[sl]),
                tri=tri,
                sbias=sbias,
            )
        )
    return maps


def run(q, k_init, v_init, attn_mask, k_stream, v_stream, trace=False, **trace_kw):
    """Run on hardware; returns (output, BassKernelResults)."""
    nc = _get_nc()
    maps = _in_maps(q, k_init, v_init, k_stream, v_stream)
    res = run_bass_kernel_spmd(nc, maps, list(range(NCORES)), trace=trace, **trace_kw)
    out = np.concatenate(
        [res.results[i]["out"].transpose(1, 0, 2, 3) for i in range(NCORES)],
        axis=1,
    ).astype(np.float32)
    return out, res


def kernel(q, k_init, v_init, attn_mask, k_stream, v_stream):
    out, _ = run(q, k_init, v_init, attn_mask, k_stream, v_stream, trace=False)
    return out
